# revision 38
# baseline (speedup 1.0000x reference)
# Multi-head masked attention (V = Q source quirk; Wv unused) on 8 TRN2 NeuronCores.
#
# Sharding: 8 cores = 4 batches x 2 query-parities. Core c handles batch b = c//2
# and the interleaved query tiles {p, p+2, p+4, ...} (p = c%2) of that batch, for
# ALL 16 heads. Each core projects K^T (all keys), Q-natural (all positions; it is
# also V due to the V=Q source bug), and Q^T for its own query half, runs causal
# attention, and produces its (disjoint) half of the output rows including the
# final projection + bias. No collectives needed; host reassembles rows.
#
# Layouts (per core, bf16 matmul operands, fp32 PSUM accumulation):
#   kT  [128=d-in-pair, HP, S]    scores lhsT  (head even: partitions 0-63)
#   qT  [128=d-in-pair, HP, Lq]   scores rhs
#   qn  [128=k-in-tile, S/128, H*(D+1)]  attnV lhsT; col D of each head slot is
#                                 a ones column -> PSUM partition 64 accumulates
#                                 the softmax denominator for free.
#   scores computed transposed (scoresT[k, q] = K @ Q^T) so the softmax sum over
#   keys is a partition-dim reduction, done by the ones column on the PE.
#   Causal masking: column-trimmed matmul/exp ranges + one data-driven frontier
#   mask multiply per (chunk, k-tile) unit; masks are per-core input data so the
#   program is SPMD-uniform across parities.

import sys

for _p in ("/opt/trn_rl_repo",):
    if _p not in sys.path:
        sys.path.append(_p)

import numpy as np
import ml_dtypes

BF16 = ml_dtypes.bfloat16

B, S, E, H = 4, 2048, 1024, 16
D = E // H
NCORES = 8

_CACHE = {}


def _unit_list(S, P=128):
    """Attention inner-loop units (qc, t, jloc) shared by builder and host."""
    Lq = S // 2
    CH = min(512, Lq)
    spc = CH // P
    n_ch = Lq // CH
    units = []
    for qc in range(n_ch):
        t_max = 2 * (qc * spc + spc - 1) + 1
        for t in range(t_max + 1):
            jstar = t // 2  # first possibly-valid local subtile (parity-1 basis)
            jloc = max(0, jstar - qc * spc)
            units.append((qc, t, jloc))
    return units, CH, spc, n_ch


def _build_program(S, E, H, n_cores=NCORES):
    import concourse.bass as bass
    import concourse.mybir as mybir
    import concourse.tile as tile
    from concourse import bacc
    from contextlib import ExitStack

    P = 128
    D = E // H
    assert D == 64 and S % 256 == 0 and E % P == 0
    S_t = S // P          # global seq tiles
    nq = S_t // 2         # local q tiles
    Lq = S // 2           # local q length
    E_t = E // P
    HP = H // 2           # head pairs
    DCH = min(512, E)     # projection d-chunk
    SCH = min(512, S)     # projection s-chunk
    QCH = min(512, Lq)    # projection q-chunk
    ECH = min(512, E)     # outproj e-chunk
    units, CH, spc, n_ch = _unit_list(S, P)
    CHB = max(CH, 512)    # per-head scores region: always a full PSUM bank
    U = len(units)
    f32 = mybir.dt.float32
    bf16 = mybir.dt.bfloat16
    Exp = mybir.ActivationFunctionType.Exp
    scale = 1.0 / float(np.sqrt(E))

    nc = bacc.Bacc(
        "TRN2", target_bir_lowering=False, debug=False, num_devices=n_cores
    )

    xT_d = nc.dram_tensor("xT", [E, S], bf16, kind="ExternalInput").ap()
    xqT_d = nc.dram_tensor("xqT", [E, Lq], bf16, kind="ExternalInput").ap()
    wqT_d = nc.dram_tensor("wqT", [E, E], bf16, kind="ExternalInput").ap()
    wkT_d = nc.dram_tensor("wkT", [E, E], bf16, kind="ExternalInput").ap()
    woT_d = nc.dram_tensor("woT", [E, E], bf16, kind="ExternalInput").ap()
    bo_d = nc.dram_tensor("bo", [1, E], bf16, kind="ExternalInput").ap()
    mask_d = nc.dram_tensor("masks", [U, P, P], bf16, kind="ExternalInput").ap()
    out_d = nc.dram_tensor("out", [Lq, E], f32, kind="ExternalOutput").ap()

    uofs = {}
    for ui, (qc, t, jloc) in enumerate(units):
        uofs[(qc, t)] = (ui, jloc)

    with tile.TileContext(nc) as tc, ExitStack() as ctx:
        main = ctx.enter_context(tc.tile_pool(name="main", bufs=1))
        expp = ctx.enter_context(tc.tile_pool(name="expp", bufs=3))
        stgp = ctx.enter_context(tc.tile_pool(name="stgp", bufs=3))
        rbfp = ctx.enter_context(tc.tile_pool(name="rbfp", bufs=3))
        ostp = ctx.enter_context(tc.tile_pool(name="ostp", bufs=2))
        pproj = ctx.enter_context(tc.tile_pool(name="pproj", bufs=2, space="PSUM"))
        psc = ctx.enter_context(tc.tile_pool(name="psc", bufs=2, space="PSUM"))
        pav = ctx.enter_context(tc.tile_pool(name="pav", bufs=1, space="PSUM"))

        kT = main.tile([P, HP, S], bf16)
        qT = main.tile([P, HP, Lq], bf16)
        qn = main.tile([P, S_t, H * (D + 1)], bf16)
        attnT = main.tile([P, HP, Lq], bf16)
        masks = main.tile([P, U, P], bf16)
        ones128 = main.tile([1, P], bf16)
        bo_sb = main.tile([1, E], bf16)
        rs = main.tile([2 * HP * n_ch, CH], bf16)    # rowsums, row=(hp,half,qc)
        rsb = main.tile([2 * HP * n_ch, CH], bf16)   # 1/rowsum

        nc.sync.dma_start(out=masks, in_=mask_d.rearrange("u p m -> p u m"))
        nc.vector.memset(ones128, 1.0)
        nc.sync.dma_start(out=bo_sb, in_=bo_d)

        qn4 = qn.rearrange("p t (h c) -> p t h c", c=D + 1)

        with tc.tile_pool(name="ph1", bufs=1) as ph1:
            xT_r = xT_d.rearrange("(t p) s -> p t s", p=P)
            xqT_r = xqT_d.rearrange("(t p) s -> p t s", p=P)
            wq_r = wqT_d.rearrange("(t p) d -> p t d", p=P)
            wk_r = wkT_d.rearrange("(t p) d -> p t d", p=P)
            xTs, xqTs = [], []
            for e in range(E_t):
                xe = ph1.tile([P, S], bf16, tag=f"xT{e}")
                xqe = ph1.tile([P, Lq], bf16, tag=f"xqT{e}")
                nc.sync.dma_start(out=xe, in_=xT_r[:, e, :])
                nc.sync.dma_start(out=xqe, in_=xqT_r[:, e, :])
                xTs.append(xe)
                xqTs.append(xqe)

            # ---- Q natural (= V) projection, head-padded with ones columns ----
            with tc.tile_pool(name="wqf", bufs=1) as wqfp:
                nh = DCH // D  # heads per d-chunk
                for dc in range(E // DCH):
                    wqfs = []
                    for e in range(E_t):
                        we = wqfp.tile([P, DCH], bf16, tag=f"wqf{e}")
                        nc.sync.dma_start(
                            out=we, in_=wq_r[:, e, dc * DCH:(dc + 1) * DCH]
                        )
                        wqfs.append(we)
                    for st in range(S_t):
                        ps = pproj.tile([P, DCH], f32)
                        for e in range(E_t):
                            nc.tensor.matmul(
                                ps,
                                xTs[e][:, st * P:(st + 1) * P],
                                wqfs[e],
                                start=(e == 0),
                                stop=(e == E_t - 1),
                            )
                        nc.vector.tensor_copy(
                            out=qn4[:, st, dc * nh:(dc + 1) * nh, 0:D],
                            in_=ps.rearrange("p (h c) -> p h c", c=D),
                        )
                        nc.vector.memset(
                            qn4[:, st, dc * nh:(dc + 1) * nh, D:D + 1], 1.0
                        )

            # ---- per head pair: K^T proj, Q^T proj, then attention ----
            with tc.tile_pool(name="whp", bufs=2) as whp:
                for hp in range(HP):
                    hA, hB = 2 * hp, 2 * hp + 1
                    wk_hp = whp.tile([P, E_t, P], bf16, tag="wk_hp")
                    wq_hp = whp.tile([P, E_t, P], bf16, tag="wq_hp")
                    for e in range(E_t):
                        nc.sync.dma_start(
                            out=wk_hp[:, e, :],
                            in_=wk_r[:, e, hp * P:(hp + 1) * P],
                        )
                        nc.sync.dma_start(
                            out=wq_hp[:, e, :],
                            in_=wq_r[:, e, hp * P:(hp + 1) * P],
                        )
                    for sc in range(S // SCH):
                        ps = pproj.tile([P, SCH], f32)
                        for e in range(E_t):
                            nc.tensor.matmul(
                                ps,
                                wk_hp[:, e, :],
                                xTs[e][:, sc * SCH:(sc + 1) * SCH],
                                start=(e == 0),
                                stop=(e == E_t - 1),
                            )
                        nc.scalar.copy(
                            out=kT[:, hp, sc * SCH:(sc + 1) * SCH], in_=ps
                        )
                    for qc2 in range(Lq // QCH):
                        ps = pproj.tile([P, QCH], f32)
                        for e in range(E_t):
                            nc.tensor.matmul(
                                ps,
                                wq_hp[:, e, :],
                                xqTs[e][:, qc2 * QCH:(qc2 + 1) * QCH],
                                start=(e == 0),
                                stop=(e == E_t - 1),
                            )
                        nc.scalar.copy(
                            out=qT[:, hp, qc2 * QCH:(qc2 + 1) * QCH], in_=ps
                        )

                    # ---- attention for this head pair ----
                    for qc in range(n_ch):
                        pvA = pav.tile([P, CH], f32)
                        pvB = pav.tile([P, CH], f32)
                        t_max = 2 * (qc * spc + spc - 1) + 1
                        for t in range(t_max + 1):
                            ui, jloc = uofs[(qc, t)]
                            qoff = jloc * P
                            sc_t = psc.tile([P, 2, CHB], f32)
                            nc.tensor.matmul(
                                sc_t[:, 0, qoff:CH],
                                kT[0:D, hp, t * P:(t + 1) * P],
                                qT[0:D, hp, qc * CH + qoff:(qc + 1) * CH],
                                start=True,
                                stop=True,
                            )
                            nc.tensor.matmul(
                                sc_t[:, 1, qoff:CH],
                                kT[D:P, hp, t * P:(t + 1) * P],
                                qT[D:P, hp, qc * CH + qoff:(qc + 1) * CH],
                                start=True,
                                stop=True,
                            )
                            ex = expp.tile([P, 2, CH], bf16)
                            nc.scalar.activation(
                                out=ex[:, :, qoff:CH],
                                in_=sc_t[:, :, qoff:CH],
                                func=Exp,
                                scale=scale,
                            )
                            for h2 in range(2):
                                nc.vector.tensor_mul(
                                    out=ex[:, h2, qoff:qoff + P],
                                    in0=ex[:, h2, qoff:qoff + P],
                                    in1=masks[:, ui, :],
                                )
                            nc.tensor.matmul(
                                pvA[0:D + 1, qoff:CH],
                                qn[:, t, hA * (D + 1):(hA + 1) * (D + 1)],
                                ex[:, 0, qoff:CH],
                                start=(t == 0),
                                stop=(t == t_max),
                            )
                            nc.tensor.matmul(
                                pvB[0:D + 1, qoff:CH],
                                qn[:, t, hB * (D + 1):(hB + 1) * (D + 1)],
                                ex[:, 1, qoff:CH],
                                start=(t == 0),
                                stop=(t == t_max),
                            )
                        # evict unnormalized attn; stage rowsum rows into rs
                        # (DVE writes must be 32-aligned -> bounce via DMA)
                        for pv, half in ((pvA, 0), (pvB, 1)):
                            row = (2 * hp + half) * n_ch + qc
                            stg = stgp.tile([1, CH], bf16)
                            nc.vector.tensor_copy(
                                out=attnT[
                                    half * D:(half + 1) * D,
                                    hp,
                                    qc * CH:(qc + 1) * CH,
                                ],
                                in_=pv[0:D, :],
                            )
                            nc.vector.tensor_copy(out=stg, in_=pv[D:D + 1, :])
                            nc.sync.dma_start(
                                out=rs[row:row + 1, :], in_=stg
                            )

        # ---- batched softmax denominators + normalization ----
        with nc.allow_low_precision("bf16 softmax denominators"):
            nc.vector.reciprocal(out=rsb, in_=rs)
        for hp in range(HP):
            for half in range(2):
                for qc in range(n_ch):
                    row = (2 * hp + half) * n_ch + qc
                    rbf = rbfp.tile([1, CH], bf16)
                    nc.sync.dma_start(out=rbf, in_=rsb[row:row + 1, :])
                    rb = pav.tile([P, CH], f32, tag="pvA" if half == 0 else "pvB")
                    nc.tensor.matmul(
                        rb[0:D, :], ones128[0:1, 0:D], rbf, start=True, stop=True
                    )
                    dst = attnT[
                        half * D:(half + 1) * D, hp, qc * CH:(qc + 1) * CH
                    ]
                    nc.vector.tensor_mul(out=dst, in0=dst, in1=rb[0:D, :])

        # ---- output projection + bias ----
        with tc.tile_pool(name="ph3", bufs=1) as ph3:
            wo_r = woT_d.rearrange("(t p) e -> p t e", p=P)
            wos = []
            for cp in range(HP):
                w1 = ph3.tile([P, E], bf16, tag=f"wo{cp}")
                nc.sync.dma_start(out=w1, in_=wo_r[:, cp, :])
                wos.append(w1)
            for st in range(nq):
                ot = ostp.tile([P, E], f32)
                for ec in range(E // ECH):
                    ps = pproj.tile([P, ECH], f32)
                    nc.tensor.matmul(
                        ps,
                        ones128[0:1, :],
                        bo_sb[0:1, ec * ECH:(ec + 1) * ECH],
                        start=True,
                        stop=False,
                    )
                    for cp in range(HP):
                        nc.tensor.matmul(
                            ps,
                            attnT[:, cp, st * P:(st + 1) * P],
                            wos[cp][:, ec * ECH:(ec + 1) * ECH],
                            start=False,
                            stop=(cp == HP - 1),
                        )
                    nc.vector.tensor_copy(out=ot[:, ec * ECH:(ec + 1) * ECH], in_=ps)
                nc.sync.dma_start(out=out_d[st * P:(st + 1) * P, :], in_=ot)

    nc.finalize()
    return nc


def _host_masks(S, parity, P=128):
    """Frontier masks per (qc, t) unit for one parity, bf16 [U, 128, 128]."""
    units, CH, spc, n_ch = _unit_list(S, P)
    CHB = max(CH, 512)  # per-head scores region: always a full PSUM bank
    tri = np.triu(np.ones((P, P), dtype=np.float32))
    ones = np.ones((P, P), dtype=np.float32)
    zeros = np.zeros((P, P), dtype=np.float32)
    out = np.empty((len(units), P, P), dtype=np.float32)
    for ui, (qc, t, jloc) in enumerate(units):
        g = 2 * (qc * spc + jloc) + parity
        if t < g:
            out[ui] = ones
        elif t == g:
            out[ui] = tri
        else:
            out[ui] = zeros
    return out.astype(BF16)


def _prep_inputs(x, Wk, Wq, Wo, bo, n_cores=NCORES):
    """Build per-core input maps (all bf16 except none; masks per parity)."""
    b, s, e = x.shape
    wqT = np.ascontiguousarray(Wq.T).astype(BF16)
    wkT = np.ascontiguousarray(Wk.T).astype(BF16)
    woT = np.ascontiguousarray(Wo.T).astype(BF16)
    bo2 = bo.reshape(1, e).astype(BF16)
    masks = [_host_masks(s, p) for p in (0, 1)]
    P = 128
    in_maps = []
    for c in range(n_cores):
        bi, p = c // 2, c % 2
        xb = x[bi]  # [S, E] f32
        xT = np.ascontiguousarray(xb.T).astype(BF16)
        qsel = xb.reshape(s // P, P, e)[p::2].reshape(s // 2, e)
        xqT = np.ascontiguousarray(qsel.T).astype(BF16)
        in_maps.append(
            {
                "xT": xT,
                "xqT": xqT,
                "wqT": wqT,
                "wkT": wkT,
                "woT": woT,
                "bo": bo2,
                "masks": masks[p],
            }
        )
    return in_maps


def kernel(x, Wk, Wq, Wv, Wo, bo):
    from concourse import bass_utils

    x = np.asarray(x, dtype=np.float32)
    Wk = np.asarray(Wk, dtype=np.float32)
    Wq = np.asarray(Wq, dtype=np.float32)
    Wo = np.asarray(Wo, dtype=np.float32)
    bo = np.asarray(bo, dtype=np.float32)
    b, s, e = x.shape
    h = H
    key = (s, e, h)
    if key not in _CACHE:
        _CACHE[key] = _build_program(s, e, h)
    nc = _CACHE[key]
    in_maps = _prep_inputs(x, Wk, Wq, Wo, bo)
    res = bass_utils.run_bass_kernel_spmd(nc, in_maps, list(range(NCORES)))
    P = 128
    out = np.empty((b, s, e), dtype=np.float32)
    for c in range(NCORES):
        bi, p = c // 2, c % 2
        oc = res.results[c]["out"]  # [Lq, E]
        out[bi].reshape(s // P, P, e)[p::2] = oc.reshape(s // 2 // P, P, e)
    return out


if __name__ == "__main__":
    # smoke: build program only
    nc = _build_program(S, E, H)
    print("built ok")
